# revision 3
# baseline (speedup 1.0000x reference)
"""Trainium2 Bass kernel for nn_MultiHeadAttention_24824910971155.

Data-parallel over batch: core b computes batch element b (B=8 == n_cores).

Per-core pipeline (all matmuls in float32r: 1 cyc/row on the PE at N=512):
  1. PE-transpose q,k,v on load (q^T etc. needed as matmul operands).
  2. Projections: QT = (q@Wq)^T, KT = (k@Wk)^T stored [D, L]; V = v@Wv stored
     natural [L, D] with a ones-column appended per head ("Vplus").
  3. Per head: S^T = KT_h^T · QT_h -> exp(S^T/8) on ScalarE -> expS^T (f32r),
     DMA'd out (the attn output, transposed + unnormalized).
  4. attn@V: o^T_h = Vplus_h^T · expS^T accumulated in PSUM; the ones column
     yields softmax row-sums for free. Row-sums DMA'd out; 1/rowsum applied to
     o^T on device (PE-replicate + DVE); qh^T residual added later from QT.
  5. o^T staged through DRAM (lane re-alignment), + QT residual, fc matmul,
     fused relu+residual(q)+rowsum via scalar_tensor_tensor, instance-norm.

Host assembles: attn[b,h,q,k] = expst[b,h,k,q] / rowsum[b,h,q].
"""

import numpy as np
from contextlib import ExitStack

B, L, D, H = 8, 1024, 1024, 16
DH = D // H          # 64
TEMP = float(DH) ** 0.5  # 8.0
EPS = 1e-6
N_CORES = 8
NT = D // 128        # 8 partition tiles
NC2 = L // 512       # 2 lq chunks

_cache = {}


def _build():
    from concourse import bacc
    import concourse.mybir as mybir
    import concourse.tile as tile
    from concourse.masks import make_identity

    F32 = mybir.dt.float32
    F32R = mybir.dt.float32r
    AF = mybir.ActivationFunctionType
    ALU = mybir.AluOpType

    nc = bacc.Bacc("TRN2", target_bir_lowering=False, debug=False)

    q_d = nc.dram_tensor("q", [L, D], F32, kind="ExternalInput")
    k_d = nc.dram_tensor("k", [L, D], F32, kind="ExternalInput")
    v_d = nc.dram_tensor("v", [L, D], F32, kind="ExternalInput")
    wq_d = nc.dram_tensor("Wq", [D, D], F32R, kind="ExternalInput")
    wk_d = nc.dram_tensor("Wk", [D, D], F32R, kind="ExternalInput")
    wv_d = nc.dram_tensor("Wv", [D, D], F32R, kind="ExternalInput")
    wfc_d = nc.dram_tensor("Wfc", [D, D], F32R, kind="ExternalInput")

    expst_d = nc.dram_tensor("expst", [H, L, L], F32R, kind="ExternalOutput")
    rowsum_d = nc.dram_tensor("rowsum", [H, L], F32R, kind="ExternalOutput")
    o_d = nc.dram_tensor("o", [L, D], F32, kind="ExternalOutput")

    oT_d = nc.dram_tensor("oT_scratch", [D, L], F32R)  # internal staging

    ts = lambda i, s: slice(i * s, (i + 1) * s)

    with tile.TileContext(nc) as tc, ExitStack() as ctx:
        constp = ctx.enter_context(tc.tile_pool(name="const", bufs=1))
        pers = ctx.enter_context(tc.tile_pool(name="pers", bufs=1))

        ident = constp.tile([128, 128], F32)
        make_identity(nc, ident[:])
        ones_grid = constp.tile([128, 128], F32)
        nc.vector.memset(ones_grid[:], 1.0)
        ones_row = constp.tile([65, 64], F32R)
        nc.vector.tensor_copy(ones_row[:], ones_grid[0:65, 0:64])

        QT = pers.tile([128, NT, L], F32R, tag="QT")

        with (
            tc.tile_pool(name="persC", bufs=1) as persC,
            tc.tile_pool(name="io", bufs=4) as iop,
        ):
            KT = persC.tile([128, NT, L], F32R, tag="KT")
            V = persC.tile([128, NT, H * (DH + 1)], F32R, tag="V")

            # ---- Stage A/B: transpose inputs + projections ----
            with (
                tc.tile_pool(name="ab", bufs=1) as ab,
                tc.tile_pool(name="abps", bufs=2, space="PSUM") as abps,
            ):
                # ones columns of Vplus: V[:, t, 65h+64] = 1.0
                v_heads = V[:].rearrange("p t (h x) -> p t h x", x=DH + 1)
                nc.vector.tensor_copy(
                    v_heads[:, :, :, DH],
                    ones_grid[:].rearrange("p (t h) -> p t h", t=NT),
                )

                for x_d, w_d, dst, kind in (
                    (q_d, wq_d, QT, "qk"),
                    (k_d, wk_d, KT, "qk"),
                    (v_d, wv_d, V, "v"),
                ):
                    xT = ab.tile([128, NT, L], F32R, tag="xT")
                    # transpose x into xT (PE transpose, 128x128 blocks)
                    for lt in range(NT):
                        x_nat = iop.tile([128, D], F32, tag="x_nat")
                        nc.sync.dma_start(x_nat[:], x_d.ap()[ts(lt, 128), :])
                        for half in range(2):
                            ps_t = abps.tile([128, 512], F32, tag="tps")
                            for j in range(4):
                                kt = 4 * half + j
                                nc.tensor.transpose(
                                    ps_t[:, ts(j, 128)], x_nat[:, ts(kt, 128)], ident[:]
                                )
                            nc.vector.tensor_copy(
                                xT[:, ts(half, 4), ts(lt, 128)],
                                ps_t[:].rearrange("p (b x) -> p b x", b=4),
                            )

                    w_sb = ab.tile([128, NT, D], F32R, tag="W")
                    nc.sync.dma_start(
                        w_sb[:], w_d.ap().rearrange("(t p) n -> p t n", p=128)
                    )

                    if kind == "qk":
                        # dst[:, nt, l] = sum_k Wx[k, nt*128+p] * x^T[k, l]
                        for nt in range(NT):
                            ps = abps.tile([128, 1024], F32, tag="pps")
                            for kt in range(NT):
                                for c in range(2):
                                    nc.tensor.matmul(
                                        ps[:, ts(c, 512)],
                                        w_sb[:, kt, ts(nt, 128)],
                                        xT[:, kt, ts(c, 512)],
                                        start=(kt == 0),
                                        stop=(kt == NT - 1),
                                    )
                            nc.vector.tensor_copy(dst[:, nt, :], ps[:])
                    else:
                        # V natural: ps[lv, n] = sum_k v[lv, k] Wv[k, n]
                        for mt in range(NT):
                            ps = abps.tile([128, 1024], F32, tag="pps")
                            for kt in range(NT):
                                for c in range(2):
                                    nc.tensor.matmul(
                                        ps[:, ts(c, 512)],
                                        xT[:, kt, ts(mt, 128)],
                                        w_sb[:, kt, ts(c, 512)],
                                        start=(kt == 0),
                                        stop=(kt == NT - 1),
                                    )
                            # scatter into Vplus head-strided columns
                            vh = V[:, mt, :].rearrange("p (h x) -> p h x", x=DH + 1)
                            for c in range(2):
                                nc.vector.tensor_copy(
                                    vh[:, ts(c, 8), 0:DH],
                                    ps[:, ts(c, 512)].rearrange(
                                        "p (h x) -> p h x", x=DH
                                    ),
                                )

            # ---- Stage C: attention per head ----
            with (
                tc.tile_pool(name="expool", bufs=12) as expool,
                tc.tile_pool(name="stgp", bufs=4) as stgp,
                tc.tile_pool(name="smalls", bufs=3) as smallp,
                tc.tile_pool(name="sps", bufs=4, space="PSUM") as spsp,
                tc.tile_pool(name="ops", bufs=2, space="PSUM") as opsp,
                tc.tile_pool(name="rps", bufs=2, space="PSUM") as rpsp,
            ):
                for h in range(H):
                    ht, po = h // 2, (h % 2) * 64
                    ex_tiles = []
                    for t in range(NT):
                        ex = expool.tile([128, L], F32R, tag="expst")
                        for c in range(NC2):
                            ps_s = spsp.tile([128, 512], F32, tag="sps")
                            nc.tensor.matmul(
                                ps_s[:],
                                KT[po : po + 64, ht, ts(t, 128)],
                                QT[po : po + 64, ht, ts(c, 512)],
                                start=True,
                                stop=True,
                            )
                            nc.scalar.activation(
                                ex[:, ts(c, 512)], ps_s[:], AF.Exp, scale=1.0 / TEMP
                            )
                        nc.sync.dma_start(expst_d.ap()[h, ts(t, 128), :], ex[:])
                        ex_tiles.append(ex)

                    for c in range(NC2):
                        ps_o = opsp.tile([65, 512], F32, tag="ops")
                        for t in range(NT):
                            nc.tensor.matmul(
                                ps_o[:],
                                V[:, t, h * (DH + 1) : (h + 1) * (DH + 1)],
                                ex_tiles[t][:, ts(c, 512)],
                                start=(t == 0),
                                stop=(t == NT - 1),
                            )
                        # rowsum (f32r) for DMA + replicate
                        rsr = smallp.tile([65, 512], F32R, tag="rsr")
                        nc.vector.tensor_copy(rsr[64:65, :], ps_o[64:65, :])
                        nc.sync.dma_start(
                            rowsum_d.ap()[h : h + 1, ts(c, 512)], rsr[64:65, :]
                        )
                        # replicate rowsum to 64 partitions via K=1 matmul
                        rep = rpsp.tile([64, 512], F32, tag="rps")
                        nc.tensor.matmul(
                            rep[:], ones_row[64:65, :], rsr[64:65, :],
                            start=True, stop=True,
                        )
                        scr = smallp.tile([64, 512], F32, tag="scr")
                        rec = smallp.tile([64, 512], F32, tag="rec")
                        nc.vector.reciprocal_approx_accurate(rec[:], rep[:], scr[:])
                        stg = stgp.tile([64, 512], F32R, tag="stg")
                        nc.vector.tensor_mul(stg[:], ps_o[0:64, :], rec[:])
                        nc.sync.dma_start(
                            oT_d.ap()[h * DH : (h + 1) * DH, ts(c, 512)], stg[:]
                        )

        # ---- Stage D: o^T + qh^T residual, fc, relu+residual+instnorm ----
        with (
            tc.tile_pool(name="dp", bufs=1) as dp,
            tc.tile_pool(name="fcs", bufs=8) as fcsp,
            tc.tile_pool(name="dsm", bufs=2) as dsm,
            tc.tile_pool(name="dps", bufs=4, space="PSUM") as dps,
        ):
            oT = dp.tile([128, NT, L], F32R, tag="oT")
            nc.sync.dma_start(oT[:], oT_d.ap().rearrange("(t p) l -> p t l", p=128))
            nc.vector.tensor_add(oT[:], oT[:], QT[:])  # + qh^T residual

            wfc = dp.tile([128, NT, D], F32R, tag="Wfc")
            nc.sync.dma_start(wfc[:], wfc_d.ap().rearrange("(t p) n -> p t n", p=128))

            sums = constp.tile([128, NT], F32)
            ssq = constp.tile([128, NT], F32)

            fc_tiles = []
            for m in range(NT):
                ps = dps.tile([128, 1024], F32, tag="fps")
                for kt in range(NT):
                    for c in range(2):
                        nc.tensor.matmul(
                            ps[:, ts(c, 512)],
                            oT[:, kt, ts(m, 128)],
                            wfc[:, kt, ts(c, 512)],
                            start=(kt == 0),
                            stop=(kt == NT - 1),
                        )
                q_nat = dsm.tile([128, D], F32, tag="q_res")
                nc.sync.dma_start(q_nat[:], q_d.ap()[ts(m, 128), :])
                fc_sb = fcsp.tile([128, D], F32, tag="fc")
                # fc_sb = relu(ps) + q ; accum rowsum for mean
                nc.vector.scalar_tensor_tensor(
                    fc_sb[:], ps[:], 0.0, q_nat[:], ALU.max, ALU.add,
                    accum_out=sums[:, m : m + 1],
                )
                sq_scr = dsm.tile([128, D], F32, tag="sq")
                nc.scalar.activation(
                    sq_scr[:], fc_sb[:], AF.Square, accum_out=ssq[:, m : m + 1]
                )
                fc_tiles.append(fc_sb)

            # instance-norm stats over D (batched across the 8 row-tiles)
            def stat(tag):
                return constp.tile([128, NT], F32, tag=tag, name=tag)
            mean, ms, mean2, var = stat("mean"), stat("ms"), stat("mean2"), stat("var")
            rcp, rstd, y2, f_t = stat("rcp"), stat("rstd"), stat("y2"), stat("f_t")
            scr8 = stat("scr8")
            nc.vector.tensor_scalar(mean[:], sums[:], 1.0 / D, None, ALU.mult)
            nc.vector.tensor_scalar(ms[:], ssq[:], 1.0 / D, None, ALU.mult)
            nc.vector.tensor_mul(mean2[:], mean[:], mean[:])
            nc.vector.tensor_sub(var[:], ms[:], mean2[:])
            nc.vector.tensor_scalar(var[:], var[:], EPS, None, ALU.add)  # var+eps
            nc.vector.reciprocal_approx_accurate(rcp[:], var[:], scr8[:])
            nc.scalar.activation(rstd[:], rcp[:], AF.Sqrt)
            # one Newton step for rstd = 1/sqrt(var+eps)
            nc.vector.tensor_mul(y2[:], rstd[:], rstd[:])
            nc.vector.tensor_mul(y2[:], var[:], y2[:])
            nc.vector.tensor_scalar(f_t[:], y2[:], -0.5, 1.5, ALU.mult, ALU.add)
            nc.vector.tensor_mul(rstd[:], rstd[:], f_t[:])

            for m in range(NT):
                o_out = dsm.tile([128, D], F32, tag="o_out")
                nc.vector.tensor_scalar(
                    o_out[:], fc_tiles[m][:],
                    mean[:, m : m + 1], rstd[:, m : m + 1],
                    ALU.subtract, ALU.mult,
                )
                nc.sync.dma_start(o_d.ap()[ts(m, 128), :], o_out[:])

    nc.compile()
    return nc


def _get_nc():
    if "nc" not in _cache:
        _cache["nc"] = _build()
    return _cache["nc"]


def run_cores(in_maps, **kw):
    """Run the SPMD kernel on cores 0..7; returns BassKernelResults."""
    from concourse.bass_utils import run_bass_kernel_spmd

    return run_bass_kernel_spmd(_get_nc(), in_maps, list(range(N_CORES)), **kw)


def kernel(q, k, v, Wq, Wk, Wv, Wfc, _trace=False, _results_out=None):
    q = np.ascontiguousarray(np.asarray(q, dtype=np.float32))
    k = np.ascontiguousarray(np.asarray(k, dtype=np.float32))
    v = np.ascontiguousarray(np.asarray(v, dtype=np.float32))
    Wq = np.ascontiguousarray(np.asarray(Wq, dtype=np.float32))
    Wk = np.ascontiguousarray(np.asarray(Wk, dtype=np.float32))
    Wv = np.ascontiguousarray(np.asarray(Wv, dtype=np.float32))
    Wfc = np.ascontiguousarray(np.asarray(Wfc, dtype=np.float32))

    in_maps = [
        {"q": q[b], "k": k[b], "v": v[b], "Wq": Wq, "Wk": Wk, "Wv": Wv, "Wfc": Wfc}
        for b in range(B)
    ]
    res = run_cores(in_maps, trace=_trace)
    if _results_out is not None:
        _results_out.append(res)

    o = np.stack([res.results[b]["o"] for b in range(B)])
    expst = np.stack([res.results[b]["expst"] for b in range(B)])  # [B,H,lk,lq]
    rowsum = np.stack([res.results[b]["rowsum"] for b in range(B)])  # [B,H,lq]
    attn = expst.transpose(0, 1, 3, 2) / rowsum[:, :, :, None]
    return o, attn
